# revision 55
# baseline (speedup 1.0000x reference)
"""Multi-head attention layer (B=2, L=S=4096, E=512, H=8, hd=64) on 8 TRN2
NeuronCores.  ~346us HW exec (prev 378us), rel err 4.5e-3.

Sharding (no collectives): core c handles batch b=c//4 and query rows
[(c%4)*1024, (c%4+1)*1024). Each core projects the full K/V of its batch
(duplicated across the 4 cores of a batch group), plus its own Q slice,
runs flash-style attention, and the output projection for its rows. Host
assembles the 8 slices.

ACT(exp) is the pacing engine: 256 x N=1024 ACTIVATEs at (N+352)/1.2 ns
= ~285us busy.  N=2048 exp is impossible on TRN2: matmul out must be fp32
(bf16 PSUM out is TRN3-only), so a score-chunk pair cannot fit 2 banks,
and 4-bank double-buffering + pv + proj exceeds the 8 PSUM banks.  PE is
the co-bottleneck (~300us of matmuls at ~215ns/N=512-MM incl hidden LDW);
mg0 is structurally PE-oversubscribed (all V/K/Q projections must finish
inside it), so scheduling targets ACT-idle:
- Scores emitted ONE CHUNK AHEAD of the exp that consumes them, and PV
  LAGGED one slot: per slot [exp(t) | scores(t+1), PV(t-1), fillers].
  The next exp's dependency is already through the in-order PE queue
  before filler bursts, and a PV that WAR-waits the previous head-pair's
  pv evacuation sits BEHIND upcoming scores instead of stalling them
  (worth ~6us: boundaries plus per-slot jitter margin).  The rest of the
  boundary normalization (rowsum/reciprocal/broadcast/mults, ~3.3us of
  DVE) lags 2 further slots so the next head-pair's filler evacuations
  are not queued behind it on DVE; only the pv evacuation itself must
  precede the new accumulator (WAR on the pv banks).
- Host-side p-major relayout of every input so each DMA is per-partition
  contiguous (2-4KB lines): q/k/v as [128, blk, ci, 512], weights
  p-major, a combined first-slice tensor w0 (wq/wk/wv head-pair-0 cols),
  ~26 coalesced issues in need-order, first K block in 256-key halves,
  w0 + half of q-mg0 on the ACT hwdge queue (parallel with SP), and the
  wq / q-mg1 weight DMAs deferred BEHIND the K/V blocks they would
  otherwise delay (needed only at t~44 / t~140).  Early single-stream
  DMA runs ~100GB/s (1 packet/engine in flight), bulk ~410GB/s; the
  ramp to t~44us is data-bound.
- Fillers (q/k/v projections) split into 2-matmul halves and placed in
  explicit (mg, hp, slot) slots by need-time and DMA arrival.  mg0-hp1
  processes chunks in ROTATED order (slot s -> chunk 10+s mod 32) so
  chunks 0-9's heads-2-7 V projections run inside hp1 itself instead of
  cramming hp0.  V heads 0-1 projected upfront from the 128KB w0 slice.
- mg1 out-proj accumulated per-head-pair into SBUF (reusing qstg's space)
  as fillers placed past the ~5us norm latency; only hp3's partial + DMA
  remain after the last exp.  Tail normalization fast path: rowsum read
  straight from PSUM, reciprocal (+bf16 cast: fp32 PE matmuls run 4x
  slow), PE ones-broadcast into PSUM, pv evacuation on the idle ACT.
- bf16 output DMA (host adds the folded bias Wo@bv + bo in f32).
- dummy exp at t=0 (ACT_TABLE_LOAD overlaps DMA wait) + 12 dummy PE
  matmuls to warm the HAM clock-gate during the initial DMA wait.
NOTE custom DVE ops silently drop the partition offset of their input AP
(rowsums are first copied to partition 0); reciprocal_approx_fast
requires fp32 out; same-pool tiles alias when a 1-buf pool rotates (a
copy into a tile aliasing its source self-deadlocks); interleaving DMA
issues of both hwdge queues deep in the program deadlocks on shared DMA
semaphore rotation (keep cross-queue DMAs to the start / disjoint sems).
"""

import numpy as np
import ml_dtypes

import concourse.bass as bass
import concourse.mybir as mybir
import concourse.tile as tile
from concourse import bacc
from concourse.bass_utils import run_bass_kernel_spmd

F32 = mybir.dt.float32
BF16 = mybir.dt.bfloat16
EXP = mybir.ActivationFunctionType.Exp
ADD = mybir.AluOpType.add
MULT = mybir.AluOpType.mult

B, L, E, H = 2, 4096, 512, 8
HD = E // H            # 64
N_CORES = 8
LLOC = B * L // N_CORES  # 1024 query rows per core
SCALE = HD ** -0.5       # 0.125

NQG = LLOC // 512   # 2 query groups of 512 rows
NSC = L // 128      # 32 key chunks of 128

_STATE = {}


def ts(i, n):
    return bass.ts(i, n)


def _build():
    nc = bacc.Bacc("TRN2", target_bir_lowering=False, debug=False,
                   num_devices=N_CORES)

    # host-prepared p-major layouts (all DMAs per-partition contiguous)
    q_d = nc.dram_tensor("qt", [128, 2, 4, 512], BF16, kind="ExternalInput")
    k_d = nc.dram_tensor("kt", [128, 8, 4, 512], BF16, kind="ExternalInput")
    v_d = nc.dram_tensor("vt", [128, 8, 4, 512], BF16, kind="ExternalInput")
    w0_d = nc.dram_tensor("w0", [128, 3, 4, 128], BF16, kind="ExternalInput")
    wq_d = nc.dram_tensor("wqf", [128, 4, 512], BF16, kind="ExternalInput")
    wk_d = nc.dram_tensor("wkf", [128, 4, 512], BF16, kind="ExternalInput")
    wv_d = nc.dram_tensor("wvf", [128, 4, 512], BF16, kind="ExternalInput")
    wo_d = nc.dram_tensor("wof", [64, 8, 512], BF16, kind="ExternalInput")
    bqk_d = nc.dram_tensor("bqk", [128, 8], F32, kind="ExternalInput")
    out_d = nc.dram_tensor("out", [E, LLOC], BF16, kind="ExternalOutput")

    with tile.TileContext(nc) as tc:
        with (
            tc.tile_pool(name="consts", bufs=1) as consts,
            tc.tile_pool(name="big", bufs=1) as big,
            tc.tile_pool(name="qstg", bufs=1) as qstg_p,
            tc.tile_pool(name="kvstg", bufs=2) as kvstg_p,
            tc.tile_pool(name="pab", bufs=3) as pab_p,
            tc.tile_pool(name="pvs", bufs=2) as pvs_p,
            tc.tile_pool(name="rv", bufs=1) as rv_p,
            tc.tile_pool(name="yt", bufs=1) as yt_p,
            tc.tile_pool(name="ps_proj", bufs=2, space="PSUM") as ps_proj,
            tc.tile_pool(name="ps_sab", bufs=2, space="PSUM") as ps_sab,
            tc.tile_pool(name="ps_pv", bufs=1, space="PSUM") as ps_pv,
        ):
            # ---------------- SBUF tiles ----------------
            w0_sb = consts.tile([128, 3, 4, 128], BF16, tag="w0")
            wq_sb = consts.tile([128, 4, E], BF16, tag="wq")
            wk_sb = consts.tile([128, 4, E], BF16, tag="wk")
            wv_sb = consts.tile([128, 4, E], BF16, tag="wv")
            wo_sb = consts.tile([64, H, E], BF16, tag="wo")
            bqk = consts.tile([128, 8], F32, tag="bqk")
            ones1 = consts.tile([1, 64], BF16, tag="ones1")
            qstg = qstg_p.tile([128, 2, 4, 512], BF16, tag="qstg")
            kstg = kvstg_p.tile([128, 8, 4, 512], BF16, tag="kv")
            vstg = kvstg_p.tile([128, 8, 4, 512], BF16, tag="kv")

            qht = big.tile([128, 4, LLOC], BF16, tag="qht")
            kht = big.tile([128, 4, L], BF16, tag="kht")
            vha = big.tile([128, NSC, H * (HD + 1)], BF16, tag="vha")
            att = big.tile([64, H, LLOC], BF16, tag="att")

            # dummy exp first: ACT_TABLE_LOAD overlaps the DMA wait
            dmy = consts.tile([128, 2], F32, tag="dmy")
            dmyo = consts.tile([128, 2], BF16, tag="dmyo")
            nc.vector.memset(dmy[:], 0.0)
            nc.scalar.activation(dmyo[:], dmy[:], EXP, scale=SCALE)

            nc.vector.memset(ones1[:], 1.0)
            # chunk-0 stripe zeroed for the PE warm-up reads; the ones
            # column and the real V projections overwrite it later
            nc.vector.memset(vha[:, 0, :], 0.0)
            nc.vector.memset(
                vha[:].rearrange("p c (h x) -> p c h x", x=HD + 1)[:, :, :, HD:HD + 1],
                1.0)

            # ---------------- DMAs: need-order, all contiguous ----------
            nc.sync.dma_start(bqk[:], bqk_d.ap())
            # w0 + half of q-mg0 go on the ACT hwdge queue: parallel with
            # the SP queue during the startup-critical window
            nc.scalar.dma_start(w0_sb[:], w0_d.ap())
            nc.scalar.dma_start(qstg[:, 0, 2:4, :], q_d.ap()[:, 0, 2:4, :])
            nc.sync.dma_start(qstg[:, 0, 0:2, :], q_d.ap()[:, 0, 0:2, :])

            def kv_block(blk):
                nc.sync.dma_start(kstg[:, blk, :, :], k_d.ap()[:, blk, :, :])
                nc.sync.dma_start(vstg[:, blk, :, :], v_d.ap()[:, blk, :, :])

            # block 0 lands in 256-key halves so scores(0) starts sooner
            nc.sync.dma_start(kstg[:, 0, :, 0:256], k_d.ap()[:, 0, :, 0:256])
            nc.sync.dma_start(kstg[:, 0, :, 256:512], k_d.ap()[:, 0, :, 256:512])
            nc.sync.dma_start(vstg[:, 0, :, :], v_d.ap()[:, 0, :, :])
            for blk in range(1, 3):
                kv_block(blk)
            nc.sync.dma_start(wv_sb[:], wv_d.ap())
            kv_block(3)
            nc.sync.dma_start(wk_sb[:], wk_d.ap())   # needed ~t=31 (k(1,*))
            kv_block(4)
            nc.sync.dma_start(wq_sb[:], wq_d.ap())   # needed ~t=44 (q(0,1))
            for blk in range(5, 8):
                kv_block(blk)
            nc.sync.dma_start(qstg[:, 1, :, :], q_d.ap()[:, 1, :, :])
            nc.sync.dma_start(wo_sb[:], wo_d.ap())

            # ---------------- projection emitters (split in halves) -----
            def q_group_parts(mg, co):
                st = {}

                def mk(cis):
                    def go():
                        if "pp" not in st:
                            st["pp"] = ps_proj.tile([128, 512], F32, tag="pp", name="pp")
                        pp = st["pp"]
                        for ci in cis:
                            lhs = (w0_sb[:, 0, ci, :] if co == 0
                                   else wq_sb[:, ci, ts(co, 128)])
                            nc.tensor.matmul(pp[:], lhs, qstg[:, mg, ci, :],
                                             start=(ci == 0), stop=(ci == 3))
                        if cis[-1] == 3:
                            nc.vector.tensor_scalar(
                                out=qht[:, co, ts(mg, 512)], in0=pp[:],
                                scalar1=bqk[:, co:co + 1], scalar2=None, op0=ADD)
                    return go
                return [mk([0, 1]), mk([2, 3])]

            def k_group_parts(hp, g):
                st = {}

                def mk(cis):
                    def go():
                        if "pp" not in st:
                            st["pp"] = ps_proj.tile([128, 512], F32, tag="pp", name="pp")
                        pp = st["pp"]
                        for ci in cis:
                            lhs = (w0_sb[:, 1, ci, :] if hp == 0
                                   else wk_sb[:, ci, ts(hp, 128)])
                            nc.tensor.matmul(pp[:], lhs, kstg[:, g, ci, :],
                                             start=(ci == 0), stop=(ci == 3))
                        if cis[-1] == 3:
                            nc.vector.tensor_scalar(
                                out=kht[:, hp, ts(g, 512)], in0=pp[:],
                                scalar1=bqk[:, 4 + hp:5 + hp], scalar2=None, op0=ADD)
                    return go
                return [mk([0, 1]), mk([2, 3])]

            def vst(sc, ci):
                return vstg[:, sc // 4, ci, ts(sc % 4, 128)]

            def v_hp0(sc):
                # project only heads 0-1 of V chunk sc (needs w0 slice)
                def go():
                    pp = ps_proj.tile([128, 512], F32, tag="pp", name="pp")
                    for ci in range(4):
                        nc.tensor.matmul(pp[:, 0:128], vst(sc, ci),
                                         w0_sb[:, 2, ci, :],
                                         start=(ci == 0), stop=(ci == 3))
                    nc.vector.tensor_copy(
                        vha[:, sc, :].rearrange("p (h x) -> p h x", x=HD + 1)[:, 0:2, 0:HD],
                        pp[:, 0:128].rearrange("p (h d) -> p h d", d=HD))
                return go

            def v_rest_parts(sc):
                # heads 2-7 of V chunk sc
                st = {}

                def mk(cis):
                    def go():
                        if "pp" not in st:
                            st["pp"] = ps_proj.tile([128, 512], F32, tag="pp", name="pp")
                        pp = st["pp"]
                        for ci in cis:
                            nc.tensor.matmul(pp[:, 0:384], vst(sc, ci),
                                             wv_sb[:, ci, 128:512],
                                             start=(ci == 0), stop=(ci == 3))
                        if cis[-1] == 3:
                            nc.vector.tensor_copy(
                                vha[:, sc, :].rearrange("p (h x) -> p h x", x=HD + 1)[:, 2:8, 0:HD],
                                pp[:, 0:384].rearrange("p (h d) -> p h d", d=HD))
                    return go
                return [mk([0, 1]), mk([2, 3])]

            def v_full_parts(sc):
                st = {}

                def mk(cis):
                    def go():
                        if "pp" not in st:
                            st["pp"] = ps_proj.tile([128, 512], F32, tag="pp", name="pp")
                        pp = st["pp"]
                        for ci in cis:
                            nc.tensor.matmul(pp[:], vst(sc, ci), wv_sb[:, ci, :],
                                             start=(ci == 0), stop=(ci == 3))
                        if cis[-1] == 3:
                            nc.vector.tensor_copy(
                                vha[:, sc, :].rearrange("p (h x) -> p h x", x=HD + 1)[:, :, 0:HD],
                                pp[:].rearrange("p (h d) -> p h d", d=HD))
                    return go
                return [mk([0, 1]), mk([2, 3])]

            def outproj0_parts(co):
                # mg0 full out-proj (runs as fillers during mg1-hp0)
                st = {}

                def mk(hs):
                    def go():
                        if "pp" not in st:
                            st["pp"] = ps_proj.tile([128, 512], F32, tag="pp", name="pp")
                        Y = st["pp"]
                        for h in hs:
                            nc.tensor.matmul(Y[:], wo_sb[:, h, ts(co, 128)],
                                             att[:, h, ts(0, 512)],
                                             start=(h == 0), stop=(h == H - 1))
                        if hs[-1] == H - 1:
                            yt = yt_p.tile([128, 512], BF16, tag="yt")
                            nc.vector.tensor_copy(yt[:], Y[:])
                            nc.sync.dma_start(out_d.ap()[ts(co, 128), ts(0, 512)], yt[:])
                    return go
                return [mk([0, 1]), mk([2, 3]), mk([4, 5]), mk([6, 7])]

            # mg1 out-proj: per-head-pair partial accumulation in SBUF.
            # yacc reuses qstg's SBUF space (qstg dead after q_group(1,*)).
            yacc_st = {}

            def yacc_tile():
                if "t" not in yacc_st:
                    yacc_st["t"] = qstg_p.tile([128, 4, 512], F32, tag="qstg", name="yacc")
                return yacc_st["t"]

            def outproj1_partial(hp, co):
                def go():
                    hA, hB = 2 * hp, 2 * hp + 1
                    yacc = yacc_tile()
                    Y = ps_proj.tile([128, 512], F32, tag="pp", name="pp")
                    nc.tensor.matmul(Y[:], wo_sb[:, hA, ts(co, 128)],
                                     att[:, hA, ts(1, 512)], start=True, stop=False)
                    nc.tensor.matmul(Y[:], wo_sb[:, hB, ts(co, 128)],
                                     att[:, hB, ts(1, 512)], start=False, stop=True)
                    if hp == 0:
                        nc.vector.tensor_copy(yacc[:, co, :], Y[:])
                    elif hp < 3:
                        nc.vector.tensor_tensor(
                            out=yacc[:, co, :], in0=yacc[:, co, :], in1=Y[:], op=ADD)
                    else:
                        if co % 2 == 0:
                            yt = yt_p.tile([128, 512], BF16, tag="yt")
                        else:
                            yt = pvs_p.tile([128, 512], BF16, tag="pvs", name="yt2")
                        nc.vector.tensor_tensor(
                            out=yt[:], in0=yacc[:, co, :], in1=Y[:], op=ADD)
                        # tail DMAs split across both hwdge queues (ACT idle)
                        eng = nc.sync if co % 2 == 0 else nc.scalar
                        eng.dma_start(out_d.ap()[ts(co, 128), ts(1, 512)], yt[:])
                return go

            # ---------------- filler schedule ----------------
            pending = {}

            def at(mg, hp, sc, fns):
                if not isinstance(fns, list):
                    fns = [fns]
                pending.setdefault((mg, hp, sc), []).extend(fns)

            # v: heads 0-1 upfront chunks 0-2; 3-9 early fillers; full-V for
            # 10..31 lead 4/3; heads 2-7: chunks 0-4 late hp0, 5-9 early hp1.
            for sc in range(3, 10):
                at(0, 0, sc - 2, v_hp0(sc))
            for sc in range(10, NSC):
                a, b = v_full_parts(sc)
                at(0, 0, sc - 4, a)
                at(0, 0, sc - 3, b)
            # mg0-hp1 is processed in rotated chunk order (slot s handles
            # chunk 10+s mod 32): chunks 0-9 are consumed at slots 22-31,
            # so their heads-2-7 V projections fit inside hp1 itself --
            # placed late (slot 16+sc, deadline 21+sc) to keep hp1's early
            # slots free for the deadline-bound k(1,*) groups
            for sc in range(10):
                at(0, 1, 16 + sc, v_rest_parts(sc))

            # k-proj: k(0,2..7) after kstg blocks land (needed slot 4g);
            # k(1,0..1) late hp0, k(1,2..7) early hp1; k(2,0) late hp1,
            # k(2,1..7) early hp2; k(3,0) late hp2, k(3,1..7) early hp3.
            karr = {2: 3, 3: 5, 4: 8, 5: 10, 6: 12, 7: 14}
            for g in range(2, 8):
                a, b = k_group_parts(0, g)
                s = max(karr[g], 4 * g - 9)
                at(0, 0, s, a)
                at(0, 0, s + 1, b)
            # rotated hp1 consumes k(1,*) groups in order 2,3,4..7,0,1:
            # the first two must finish in hp0, the rest ride hp1 odd slots
            for g in range(2, 4):
                a, b = k_group_parts(1, g)
                at(0, 0, 12 + 2 * g, a)         # g=2 -> 16,17; g=3 -> 18,19
                at(0, 0, 13 + 2 * g, b)
            for i, g in enumerate([4, 5, 6, 7]):
                a, b = k_group_parts(1, g)
                at(0, 1, 4 * i + 1, a)
                at(0, 1, 4 * i + 3, b)
            for i, g in enumerate([0, 1]):      # deadlines: slots 20 / 24
                a, b = k_group_parts(1, g)
                at(0, 1, 4 * i + 2, a)
                at(0, 1, 4 * i + 4, b)
            a, b = k_group_parts(2, 0)
            at(0, 1, 24, a)
            at(0, 1, 26, b)
            for g in range(1, 8):
                a, b = k_group_parts(2, g)
                at(0, 2, 2 * g - 2, a)
                at(0, 2, 2 * g - 1, b)
            a, b = k_group_parts(3, 0)
            at(0, 2, 26, a)
            at(0, 2, 27, b)
            for g in range(1, 8):
                a, b = k_group_parts(3, g)
                at(0, 3, 2 * g - 2, a)
                at(0, 3, 2 * g - 1, b)

            # q-proj: q(0,1) late hp0, q(0,2) mid hp1, q(0,3) mid hp2;
            # q(1,*) during hp3 (interleaved with k(3,*))
            a, b = q_group_parts(0, 1)
            at(0, 0, 29, a)
            at(0, 0, 30, b)
            a, b = q_group_parts(0, 2)
            at(0, 1, 25, a)
            at(0, 1, 27, b)
            a, b = q_group_parts(0, 3)
            at(0, 2, 20, [a, b])
            # q(1,0) must land in mg0-hp3 (mg1-hp0 scores emitted at its
            # last slot); q(1,1..3) ride in mg1-hp0 (needed at hp1/2/3)
            a, b = q_group_parts(1, 0)
            at(0, 3, 15, a)
            at(0, 3, 16, b)
            for co in range(1, 4):
                a, b = q_group_parts(1, co)
                at(1, 0, 8 * co - 6, [a, b])

            # mg1 fillers: mg0 out-proj during hp0; partial out-proj placed
            # past the ~5us normalization latency of the previous head-pair
            # so its matmuls never head-block the PE queue
            for co in range(4):
                for i, fn in enumerate(outproj0_parts(co)):
                    at(1, 0, 8 * co + 2 * i + 1, fn)
            for hp in range(3):
                for co in range(4):
                    at(1, hp + 1, 8 + 4 * co, outproj1_partial(hp, co))

            # ---------------- PE warm-up ----------------
            # ~5us of dummy matmuls while the input DMA streams: drives the
            # HAM clock-gate to 8/8 so the first projections run at 2.4GHz.
            # Alternate psum pools for 4-deep rotation (back-to-back MMs).
            for i in range(12):
                if i % 2 == 0:
                    wup = ps_proj.tile([128, 512], F32, tag="pp", name="wup")
                    dst = wup[0:64, :]
                else:
                    wup = ps_sab.tile([128, 2, 512], F32, tag="sab", name="wup")
                    dst = wup[0:64, 0, :]
                nc.tensor.matmul(dst, vha[0:64, 0, 0:64],
                                 vha[0:64, 0, 0:512], start=True, stop=True)

            # ---------------- upfront projections ----------------
            for fn in q_group_parts(0, 0):
                fn()
            # k(0,0) in 256-key halves, tracking the split block-0 DMA
            for half in range(2):
                pp = ps_proj.tile([128, 512], F32, tag="pp", name="pp")
                for ci in range(4):
                    nc.tensor.matmul(pp[:, ts(half, 256)],
                                     w0_sb[:, 1, ci, :],
                                     kstg[:, 0, ci, ts(half, 256)],
                                     start=(ci == 0), stop=(ci == 3))
                nc.vector.tensor_scalar(
                    out=kht[:, 0, ts(half, 256)], in0=pp[:, ts(half, 256)],
                    scalar1=bqk[:, 4:5], scalar2=None, op0=ADD)
            for fn in k_group_parts(0, 1):
                fn()
            for sc in range(3):
                v_hp0(sc)()

            # ---------------- attention (flat, scores one chunk ahead) ----
            NT = NQG * 4 * NSC
            ROT = {(0, 1): 10}   # mg0-hp1: slot s handles chunk 10+s mod 32

            def decode(t):
                return t // (4 * NSC), (t // NSC) % 4, t % NSC

            def chunk_of(mg, hp, s):
                return (s + ROT.get((mg, hp), 0)) % NSC

            def emit_scores(t):
                mg, hp, s = decode(t)
                sc = chunk_of(mg, hp, s)
                sab = ps_sab.tile([128, 2, 512], F32, tag="sab")
                nc.tensor.matmul(sab[:, 0, :],
                                 kht[0:64, hp, ts(sc, 128)],
                                 qht[0:64, hp, ts(mg, 512)],
                                 start=True, stop=True, tile_position=(0, 0))
                nc.tensor.matmul(sab[:, 1, :],
                                 kht[64:128, hp, ts(sc, 128)],
                                 qht[64:128, hp, ts(mg, 512)],
                                 start=True, stop=True, tile_position=(64, 0))
                return sab

            pab_hist = {}
            pv_of = {}

            def emit_pv(tt):
                mg_, hp_, s_ = decode(tt)
                ch_ = chunk_of(mg_, hp_, s_)
                hA_, hB_ = 2 * hp_, 2 * hp_ + 1
                pvt = pv_of.pop(tt)
                pab_ = pab_hist.pop(tt)
                nc.tensor.matmul(pvt[:, 0, :],
                                 vha[:, ch_, hA_ * 65: hA_ * 65 + 65],
                                 pab_[:, 0, :],
                                 start=(s_ == 0), stop=(s_ == NSC - 1))
                nc.tensor.matmul(pvt[:, 1, :],
                                 vha[:, ch_, hB_ * 65: hB_ * 65 + 65],
                                 pab_[:, 1, :],
                                 start=(s_ == 0), stop=(s_ == NSC - 1))
                return pvt

            def emit_evac(pv_):
                # only the PSUM evacuation must precede the next
                # head-pair's first PV (WAR on the pv banks)
                pvs = pvs_p.tile([65, 2, 512], F32, tag="pvs")
                nc.vector.tensor_copy(pvs[:], pv_[:])
                return pvs

            def emit_norm_rest(mg_, hp_, pvs):
                # SBUF-side normalization: lagged 2 slots so the boundary
                # fillers' DVE evacuations are not queued behind it
                hA_, hB_ = 2 * hp_, 2 * hp_ + 1
                rs = rv_p.tile([1, 2, 512], F32, tag="rs")
                nc.vector.tensor_copy(rs[:], pvs[64:65, :, :])
                rv = rv_p.tile([1, 2, 512], F32, tag="rv")
                nc.vector.reciprocal_approx_fast(out=rv[:], in_=rs[:])
                rrep = pvs_p.tile([64, 2, 512], F32, tag="pvs", name="rrep")
                nc.gpsimd.partition_broadcast(rrep[:], rv[:])
                for i, h in ((0, hA_), (1, hB_)):
                    nc.vector.tensor_tensor(
                        out=att[:, h, ts(mg_, 512)], in0=pvs[0:64, i, :],
                        in1=rrep[:, i, :], op=MULT)

            def emit_norm(mg_, hp_, pv_, is_last):
                hA_, hB_ = 2 * hp_, 2 * hp_ + 1
                if not is_last:
                    emit_norm_rest(mg_, hp_, emit_evac(pv_))
                else:
                    # tail fast path: rowsum straight from PSUM,
                    # reciprocal, PE ones-broadcast into PSUM, multiply.
                    # pv evacuation runs on the (now idle) ACT engine in
                    # parallel with the DVE rowsum/reciprocal chain.
                    rs = rv_p.tile([1, 2, 512], F32, tag="rs")
                    nc.vector.tensor_copy(rs[:], pv_[64:65, :, :])
                    pvs = pvs_p.tile([65, 2, 512], F32, tag="pvs")
                    nc.scalar.copy(pvs[:], pv_[:])
                    rv = rv_p.tile([1, 2, 512], F32, tag="rv")
                    nc.vector.reciprocal_approx_fast(out=rv[:], in_=rs[:])
                    rvb = pvs_p.tile([1, 2, 512], BF16, tag="pvs", name="rvb")
                    nc.vector.tensor_copy(rvb[:], rv[:])
                    bc = ps_sab.tile([64, 2, 512], F32, tag="sab", name="bc")
                    for i in range(2):
                        nc.tensor.matmul(bc[:, i, :], ones1[:, :],
                                         rvb[:, i, :], start=True, stop=True)
                    for i, h in ((0, hA_), (1, hB_)):
                        nc.vector.tensor_tensor(
                            out=att[:, h, ts(mg_, 512)], in0=pvs[0:64, i, :],
                            in1=bc[:, i, :], op=MULT)

            # PV emission lags one slot: slot t emits [exp(t), scores(t+1),
            # pv(t-1), fillers].  At head-pair boundaries the new
            # accumulator's first PV (which WAR-waits the previous pv's
            # DVE evacuation) then sits BEHIND the next chunks' scores in
            # the in-order PE queue instead of stalling them.
            sab_cur = emit_scores(0)
            pv_cur = ps_pv.tile([65, 2, 512], F32, tag="pv")
            norm_pending = None
            for t in range(NT):
                mg, hp, sc = decode(t)
                last = (t == NT - 1)
                pab = pab_p.tile([128, 2, 512], BF16, tag="pab")
                pab_hist[t] = pab
                nc.scalar.activation(pab[:], sab_cur[:], EXP, scale=SCALE)
                if not last:
                    sab_cur = emit_scores(t + 1)
                if t >= 1:
                    pv_done = emit_pv(t - 1)
                    if sc == 0:
                        # t-1 closed the previous head-pair: evacuate its
                        # pv now, allocate this head-pair's accumulator;
                        # the rest of the normalization lags 2 slots
                        pmg, php, _ = decode(t - 1)
                        norm_pending = (pmg, php, emit_evac(pv_done))
                        pv_cur = ps_pv.tile([65, 2, 512], F32, tag="pv")
                if sc == 2 and norm_pending is not None:
                    emit_norm_rest(*norm_pending)
                    norm_pending = None
                pv_of[t] = pv_cur
                for fn in pending.pop((mg, hp, sc), ()):
                    fn()
            pv_done = emit_pv(NT - 1)
            emit_norm(NQG - 1, 3, pv_done, is_last=True)

            # ---------------- tail: last head-pair partials ----------------
            for co in range(4):
                outproj1_partial(3, co)()

            assert not pending, f"unconsumed fillers: {list(pending)}"

    nc.compile()
    return nc


def _get_nc():
    if "nc" not in _STATE:
        _STATE["nc"] = _build()
    return _STATE["nc"]


def _bf16(x):
    return np.ascontiguousarray(x.astype(ml_dtypes.bfloat16))


def _pmajor(xT):
    # xT: [E, Lc] (feature-major) -> [128, Lc//512, 4, 512]
    # out[p, blk, ci, l] = xT[ci*128 + p, blk*512 + l]
    E_, Lc = xT.shape
    return np.ascontiguousarray(
        xT.reshape(4, 128, Lc // 512, 512).transpose(1, 2, 0, 3))


def _wmajor(WT):
    # WT: [E, E] -> [128, 4, 512]; out[p, ci, o] = WT[ci*128 + p, o]
    return np.ascontiguousarray(WT.reshape(4, 128, 512).transpose(1, 0, 2))


def _shard(inputs):
    q = np.asarray(inputs["q"], dtype=np.float32)
    k = np.asarray(inputs["k"], dtype=np.float32)
    v = np.asarray(inputs["v"], dtype=np.float32)
    WqT = np.asarray(inputs["Wq"], np.float32).T
    WkT = np.asarray(inputs["Wk"], np.float32).T
    WvT = np.asarray(inputs["Wv"], np.float32).T
    WoT = np.asarray(inputs["Wo"], np.float32).T
    bq = np.asarray(inputs["bq"], np.float32)
    bk = np.asarray(inputs["bk"], np.float32)

    wqf = _bf16(_wmajor(WqT))
    wkf = _bf16(_wmajor(WkT))
    wvf = _bf16(_wmajor(WvT))
    # w0: first-128-output-cols slices of wq/wk/wv: [128, 3, 4, 128]
    w0 = _bf16(np.stack([wqf[:, :, 0:128], wkf[:, :, 0:128],
                         wvf[:, :, 0:128]], axis=1))
    # wo: [64, 8, 512]; [d, h, o] = WoT[h*64+d, o]
    wof = _bf16(WoT.reshape(8, 64, 512).transpose(1, 0, 2))
    bqk = np.ascontiguousarray(
        np.concatenate([bq.reshape(4, 128).T, bk.reshape(4, 128).T], axis=1))

    kt = [_bf16(_pmajor(k[b].T)) for b in range(B)]
    vt = [_bf16(_pmajor(v[b].T)) for b in range(B)]

    in_maps = []
    for c in range(N_CORES):
        b, j = divmod(c, N_CORES // B)
        in_maps.append({
            "qt": _bf16(_pmajor(q[b, j * LLOC:(j + 1) * LLOC].T)),
            "kt": kt[b],
            "vt": vt[b],
            "w0": w0, "wqf": wqf, "wkf": wkf, "wvf": wvf, "wof": wof,
            "bqk": bqk,
        })
    return in_maps


def _run(inputs, trace=False):
    nc = _get_nc()
    in_maps = _shard(inputs)
    res = run_bass_kernel_spmd(nc, in_maps, core_ids=list(range(N_CORES)),
                               trace=trace)
    # v-bias commutes through attention (rows of P sum to 1 after
    # normalization): fold Wo @ bv into the output bias, added on host.
    Wo = np.asarray(inputs["Wo"], np.float32)
    bo_eff = (np.asarray(inputs["bo"], np.float32)
              + Wo @ np.asarray(inputs["bv"], np.float32))
    out = np.empty((B, L, E), np.float32)
    for c in range(N_CORES):
        b, j = divmod(c, N_CORES // B)
        out[b, j * LLOC:(j + 1) * LLOC] = \
            res.results[c]["out"].astype(np.float32).T + bo_eff
    return out, res


def kernel(**inputs) -> np.ndarray:
    return _run(inputs)[0]


# revision 56
# speedup vs baseline: 1.0073x; 1.0073x over previous
"""Multi-head attention layer (B=2, L=S=4096, E=512, H=8, hd=64) on 8 TRN2
NeuronCores.  ~346us HW exec (prev 378us), rel err 4.5e-3.

Sharding (no collectives): core c handles batch b=c//4 and query rows
[(c%4)*1024, (c%4+1)*1024). Each core projects the full K/V of its batch
(duplicated across the 4 cores of a batch group), plus its own Q slice,
runs flash-style attention, and the output projection for its rows. Host
assembles the 8 slices.

ACT(exp) is the pacing engine: 256 x N=1024 ACTIVATEs at (N+352)/1.2 ns
= ~285us busy.  N=2048 exp is impossible on TRN2: matmul out must be fp32
(bf16 PSUM out is TRN3-only), so a score-chunk pair cannot fit 2 banks,
and 4-bank double-buffering + pv + proj exceeds the 8 PSUM banks.  PE is
the co-bottleneck (~300us of matmuls at ~215ns/N=512-MM incl hidden LDW);
mg0 is structurally PE-oversubscribed (all V/K/Q projections must finish
inside it), so scheduling targets ACT-idle:
- Scores emitted ONE CHUNK AHEAD of the exp that consumes them, and PV
  LAGGED one slot: per slot [exp(t) | scores(t+1), PV(t-1), fillers].
  The next exp's dependency is already through the in-order PE queue
  before filler bursts, and a PV that WAR-waits the previous head-pair's
  pv evacuation sits BEHIND upcoming scores instead of stalling them
  (worth ~6us: boundaries plus per-slot jitter margin).  The rest of the
  boundary normalization (rowsum/reciprocal/broadcast/mults, ~3.3us of
  DVE) lags 2 further slots so the next head-pair's filler evacuations
  are not queued behind it on DVE; only the pv evacuation itself must
  precede the new accumulator (WAR on the pv banks).
- Host-side p-major relayout of every input so each DMA is per-partition
  contiguous (2-4KB lines): q/k/v as [128, blk, ci, 512], weights
  p-major, a combined first-slice tensor w0 (wq/wk/wv head-pair-0 cols),
  ~26 coalesced issues in need-order, first K block in 256-key halves,
  w0 + half of q-mg0 on the ACT hwdge queue (parallel with SP), and the
  wq / q-mg1 weight DMAs deferred BEHIND the K/V blocks they would
  otherwise delay (needed only at t~44 / t~140).  Early single-stream
  DMA runs ~100GB/s (1 packet/engine in flight), bulk ~410GB/s; the
  ramp to t~44us is data-bound.
- Fillers (q/k/v projections) split into 2-matmul halves and placed in
  explicit (mg, hp, slot) slots by need-time and DMA arrival.  mg0-hp1
  processes chunks in ROTATED order (slot s -> chunk 10+s mod 32) so
  chunks 0-9's heads-2-7 V projections run inside hp1 itself instead of
  cramming hp0.  V heads 0-1 projected upfront from the 128KB w0 slice.
- mg1 out-proj accumulated per-head-pair into SBUF (reusing qstg's space)
  as fillers placed past the ~5us norm latency; only hp3's partial + DMA
  remain after the last exp.  Tail normalization fast path: rowsum read
  straight from PSUM, reciprocal (+bf16 cast: fp32 PE matmuls run 4x
  slow), PE ones-broadcast into PSUM, pv evacuation on the idle ACT.
- bf16 output DMA (host adds the folded bias Wo@bv + bo in f32).
- dummy exp at t=0 (ACT_TABLE_LOAD overlaps DMA wait) + 12 dummy PE
  matmuls to warm the HAM clock-gate during the initial DMA wait.
NOTE custom DVE ops silently drop the partition offset of their input AP
(rowsums are first copied to partition 0); reciprocal_approx_fast
requires fp32 out; same-pool tiles alias when a 1-buf pool rotates (a
copy into a tile aliasing its source self-deadlocks); interleaving DMA
issues of both hwdge queues deep in the program deadlocks on shared DMA
semaphore rotation (keep cross-queue DMAs to the start / disjoint sems).
"""

import numpy as np
import ml_dtypes

import concourse.bass as bass
import concourse.mybir as mybir
import concourse.tile as tile
from concourse import bacc
from concourse.bass_utils import run_bass_kernel_spmd

F32 = mybir.dt.float32
BF16 = mybir.dt.bfloat16
EXP = mybir.ActivationFunctionType.Exp
ADD = mybir.AluOpType.add
MULT = mybir.AluOpType.mult

B, L, E, H = 2, 4096, 512, 8
HD = E // H            # 64
N_CORES = 8
LLOC = B * L // N_CORES  # 1024 query rows per core
SCALE = HD ** -0.5       # 0.125

NQG = LLOC // 512   # 2 query groups of 512 rows
NSC = L // 128      # 32 key chunks of 128

_STATE = {}


def ts(i, n):
    return bass.ts(i, n)


def _build():
    nc = bacc.Bacc("TRN2", target_bir_lowering=False, debug=False,
                   num_devices=N_CORES)

    # host-prepared p-major layouts (all DMAs per-partition contiguous)
    q_d = nc.dram_tensor("qt", [128, 2, 4, 512], BF16, kind="ExternalInput")
    k_d = nc.dram_tensor("kt", [128, 8, 4, 512], BF16, kind="ExternalInput")
    v_d = nc.dram_tensor("vt", [128, 8, 4, 512], BF16, kind="ExternalInput")
    w0_d = nc.dram_tensor("w0", [128, 3, 4, 128], BF16, kind="ExternalInput")
    wq_d = nc.dram_tensor("wqf", [128, 4, 512], BF16, kind="ExternalInput")
    wk_d = nc.dram_tensor("wkf", [128, 4, 512], BF16, kind="ExternalInput")
    wv_d = nc.dram_tensor("wvf", [128, 4, 512], BF16, kind="ExternalInput")
    wo_d = nc.dram_tensor("wof", [64, 8, 512], BF16, kind="ExternalInput")
    bqk_d = nc.dram_tensor("bqk", [128, 8], F32, kind="ExternalInput")
    out_d = nc.dram_tensor("out", [E, LLOC], BF16, kind="ExternalOutput")

    with tile.TileContext(nc) as tc:
        with (
            tc.tile_pool(name="consts", bufs=1) as consts,
            tc.tile_pool(name="big", bufs=1) as big,
            tc.tile_pool(name="qstg", bufs=1) as qstg_p,
            tc.tile_pool(name="kvstg", bufs=2) as kvstg_p,
            tc.tile_pool(name="pab", bufs=3) as pab_p,
            tc.tile_pool(name="pvs", bufs=2) as pvs_p,
            tc.tile_pool(name="rv", bufs=1) as rv_p,
            tc.tile_pool(name="yt", bufs=1) as yt_p,
            tc.tile_pool(name="ps_proj", bufs=2, space="PSUM") as ps_proj,
            tc.tile_pool(name="ps_sab", bufs=2, space="PSUM") as ps_sab,
            tc.tile_pool(name="ps_pv", bufs=1, space="PSUM") as ps_pv,
        ):
            # ---------------- SBUF tiles ----------------
            w0_sb = consts.tile([128, 3, 4, 128], BF16, tag="w0")
            wq_sb = consts.tile([128, 4, E], BF16, tag="wq")
            wk_sb = consts.tile([128, 4, E], BF16, tag="wk")
            wv_sb = consts.tile([128, 4, E], BF16, tag="wv")
            wo_sb = consts.tile([64, H, E], BF16, tag="wo")
            bqk = consts.tile([128, 8], F32, tag="bqk")
            ones1 = consts.tile([1, 64], BF16, tag="ones1")
            qstg = qstg_p.tile([128, 2, 4, 512], BF16, tag="qstg")
            kstg = kvstg_p.tile([128, 8, 4, 512], BF16, tag="kv")
            vstg = kvstg_p.tile([128, 8, 4, 512], BF16, tag="kv")

            qht = big.tile([128, 4, LLOC], BF16, tag="qht")
            kht = big.tile([128, 4, L], BF16, tag="kht")
            vha = big.tile([128, NSC, H * (HD + 1)], BF16, tag="vha")
            att = big.tile([64, H, LLOC], BF16, tag="att")

            # dummy exp first: ACT_TABLE_LOAD overlaps the DMA wait
            dmy = consts.tile([128, 2], F32, tag="dmy")
            dmyo = consts.tile([128, 2], BF16, tag="dmyo")
            nc.vector.memset(dmy[:], 0.0)
            nc.scalar.activation(dmyo[:], dmy[:], EXP, scale=SCALE)

            nc.vector.memset(ones1[:], 1.0)
            # chunk-0 stripe zeroed for the PE warm-up reads; the ones
            # column and the real V projections overwrite it later
            nc.vector.memset(vha[:, 0, :], 0.0)
            nc.vector.memset(
                vha[:].rearrange("p c (h x) -> p c h x", x=HD + 1)[:, :, :, HD:HD + 1],
                1.0)

            # ---------------- DMAs: need-order, all contiguous ----------
            # w0 + half of q-mg0 go on the ACT hwdge queue: parallel with
            # the SP queue during the startup-critical window
            nc.scalar.dma_start(w0_sb[:], w0_d.ap())
            nc.scalar.dma_start(qstg[:, 0, 2:4, :], q_d.ap()[:, 0, 2:4, :])
            nc.sync.dma_start(qstg[:, 0, 0:2, :], q_d.ap()[:, 0, 0:2, :])

            def kv_block(blk):
                nc.sync.dma_start(kstg[:, blk, :, :], k_d.ap()[:, blk, :, :])
                nc.sync.dma_start(vstg[:, blk, :, :], v_d.ap()[:, blk, :, :])

            # block 0 lands in 256-key halves so scores(0) starts sooner
            nc.sync.dma_start(kstg[:, 0, :, 0:256], k_d.ap()[:, 0, :, 0:256])
            nc.sync.dma_start(kstg[:, 0, :, 256:512], k_d.ap()[:, 0, :, 256:512])
            nc.sync.dma_start(vstg[:, 0, :, :], v_d.ap()[:, 0, :, :])
            # biases are tiny and first needed ~t=16: keep their issue off
            # the SP queue's startup-critical head slot
            nc.sync.dma_start(bqk[:], bqk_d.ap())
            for blk in range(1, 3):
                kv_block(blk)
            nc.sync.dma_start(wv_sb[:], wv_d.ap())
            kv_block(3)
            nc.sync.dma_start(wk_sb[:], wk_d.ap())   # needed ~t=31 (k(1,*))
            kv_block(4)
            nc.sync.dma_start(wq_sb[:], wq_d.ap())   # needed ~t=44 (q(0,1))
            for blk in range(5, 8):
                kv_block(blk)
            nc.sync.dma_start(qstg[:, 1, :, :], q_d.ap()[:, 1, :, :])
            nc.sync.dma_start(wo_sb[:], wo_d.ap())

            # ---------------- projection emitters (split in halves) -----
            def q_group_parts(mg, co):
                st = {}

                def mk(cis):
                    def go():
                        if "pp" not in st:
                            st["pp"] = ps_proj.tile([128, 512], F32, tag="pp", name="pp")
                        pp = st["pp"]
                        for ci in cis:
                            lhs = (w0_sb[:, 0, ci, :] if co == 0
                                   else wq_sb[:, ci, ts(co, 128)])
                            nc.tensor.matmul(pp[:], lhs, qstg[:, mg, ci, :],
                                             start=(ci == 0), stop=(ci == 3))
                        if cis[-1] == 3:
                            nc.vector.tensor_scalar(
                                out=qht[:, co, ts(mg, 512)], in0=pp[:],
                                scalar1=bqk[:, co:co + 1], scalar2=None, op0=ADD)
                    return go
                return [mk([0, 1]), mk([2, 3])]

            def k_group_parts(hp, g):
                st = {}

                def mk(cis):
                    def go():
                        if "pp" not in st:
                            st["pp"] = ps_proj.tile([128, 512], F32, tag="pp", name="pp")
                        pp = st["pp"]
                        for ci in cis:
                            lhs = (w0_sb[:, 1, ci, :] if hp == 0
                                   else wk_sb[:, ci, ts(hp, 128)])
                            nc.tensor.matmul(pp[:], lhs, kstg[:, g, ci, :],
                                             start=(ci == 0), stop=(ci == 3))
                        if cis[-1] == 3:
                            nc.vector.tensor_scalar(
                                out=kht[:, hp, ts(g, 512)], in0=pp[:],
                                scalar1=bqk[:, 4 + hp:5 + hp], scalar2=None, op0=ADD)
                    return go
                return [mk([0, 1]), mk([2, 3])]

            def vst(sc, ci):
                return vstg[:, sc // 4, ci, ts(sc % 4, 128)]

            def v_hp0(sc):
                # project only heads 0-1 of V chunk sc (needs w0 slice)
                def go():
                    pp = ps_proj.tile([128, 512], F32, tag="pp", name="pp")
                    for ci in range(4):
                        nc.tensor.matmul(pp[:, 0:128], vst(sc, ci),
                                         w0_sb[:, 2, ci, :],
                                         start=(ci == 0), stop=(ci == 3))
                    nc.vector.tensor_copy(
                        vha[:, sc, :].rearrange("p (h x) -> p h x", x=HD + 1)[:, 0:2, 0:HD],
                        pp[:, 0:128].rearrange("p (h d) -> p h d", d=HD))
                return go

            def v_rest_parts(sc):
                # heads 2-7 of V chunk sc
                st = {}

                def mk(cis):
                    def go():
                        if "pp" not in st:
                            st["pp"] = ps_proj.tile([128, 512], F32, tag="pp", name="pp")
                        pp = st["pp"]
                        for ci in cis:
                            nc.tensor.matmul(pp[:, 0:384], vst(sc, ci),
                                             wv_sb[:, ci, 128:512],
                                             start=(ci == 0), stop=(ci == 3))
                        if cis[-1] == 3:
                            nc.vector.tensor_copy(
                                vha[:, sc, :].rearrange("p (h x) -> p h x", x=HD + 1)[:, 2:8, 0:HD],
                                pp[:, 0:384].rearrange("p (h d) -> p h d", d=HD))
                    return go
                return [mk([0, 1]), mk([2, 3])]

            def v_full_parts(sc):
                st = {}

                def mk(cis):
                    def go():
                        if "pp" not in st:
                            st["pp"] = ps_proj.tile([128, 512], F32, tag="pp", name="pp")
                        pp = st["pp"]
                        for ci in cis:
                            nc.tensor.matmul(pp[:], vst(sc, ci), wv_sb[:, ci, :],
                                             start=(ci == 0), stop=(ci == 3))
                        if cis[-1] == 3:
                            nc.vector.tensor_copy(
                                vha[:, sc, :].rearrange("p (h x) -> p h x", x=HD + 1)[:, :, 0:HD],
                                pp[:].rearrange("p (h d) -> p h d", d=HD))
                    return go
                return [mk([0, 1]), mk([2, 3])]

            def outproj0_parts(co):
                # mg0 full out-proj (runs as fillers during mg1-hp0)
                st = {}

                def mk(hs):
                    def go():
                        if "pp" not in st:
                            st["pp"] = ps_proj.tile([128, 512], F32, tag="pp", name="pp")
                        Y = st["pp"]
                        for h in hs:
                            nc.tensor.matmul(Y[:], wo_sb[:, h, ts(co, 128)],
                                             att[:, h, ts(0, 512)],
                                             start=(h == 0), stop=(h == H - 1))
                        if hs[-1] == H - 1:
                            yt = yt_p.tile([128, 512], BF16, tag="yt")
                            nc.vector.tensor_copy(yt[:], Y[:])
                            nc.sync.dma_start(out_d.ap()[ts(co, 128), ts(0, 512)], yt[:])
                    return go
                return [mk([0, 1]), mk([2, 3]), mk([4, 5]), mk([6, 7])]

            # mg1 out-proj: per-head-pair partial accumulation in SBUF.
            # yacc reuses qstg's SBUF space (qstg dead after q_group(1,*)).
            yacc_st = {}

            def yacc_tile():
                if "t" not in yacc_st:
                    yacc_st["t"] = qstg_p.tile([128, 4, 512], F32, tag="qstg", name="yacc")
                return yacc_st["t"]

            def outproj1_partial(hp, co):
                def go():
                    hA, hB = 2 * hp, 2 * hp + 1
                    yacc = yacc_tile()
                    Y = ps_proj.tile([128, 512], F32, tag="pp", name="pp")
                    nc.tensor.matmul(Y[:], wo_sb[:, hA, ts(co, 128)],
                                     att[:, hA, ts(1, 512)], start=True, stop=False)
                    nc.tensor.matmul(Y[:], wo_sb[:, hB, ts(co, 128)],
                                     att[:, hB, ts(1, 512)], start=False, stop=True)
                    if hp == 0:
                        nc.vector.tensor_copy(yacc[:, co, :], Y[:])
                    elif hp < 3:
                        nc.vector.tensor_tensor(
                            out=yacc[:, co, :], in0=yacc[:, co, :], in1=Y[:], op=ADD)
                    else:
                        if co % 2 == 0:
                            yt = yt_p.tile([128, 512], BF16, tag="yt")
                        else:
                            yt = pvs_p.tile([128, 512], BF16, tag="pvs", name="yt2")
                        nc.vector.tensor_tensor(
                            out=yt[:], in0=yacc[:, co, :], in1=Y[:], op=ADD)
                        # tail DMAs split across both hwdge queues (ACT idle)
                        eng = nc.sync if co % 2 == 0 else nc.scalar
                        eng.dma_start(out_d.ap()[ts(co, 128), ts(1, 512)], yt[:])
                return go

            # ---------------- filler schedule ----------------
            pending = {}

            def at(mg, hp, sc, fns):
                if not isinstance(fns, list):
                    fns = [fns]
                pending.setdefault((mg, hp, sc), []).extend(fns)

            # v: heads 0-1 upfront chunks 0-2; 3-9 early fillers; full-V for
            # 10..31 lead 4/3; heads 2-7: chunks 0-4 late hp0, 5-9 early hp1.
            for sc in range(3, 10):
                at(0, 0, sc - 2, v_hp0(sc))
            for sc in range(10, NSC):
                a, b = v_full_parts(sc)
                at(0, 0, sc - 4, a)
                at(0, 0, sc - 3, b)
            # mg0-hp1 is processed in rotated chunk order (slot s handles
            # chunk 10+s mod 32): chunks 0-9 are consumed at slots 22-31,
            # so their heads-2-7 V projections fit inside hp1 itself --
            # placed late (slot 16+sc, deadline 21+sc) to keep hp1's early
            # slots free for the deadline-bound k(1,*) groups
            for sc in range(10):
                at(0, 1, 16 + sc, v_rest_parts(sc))

            # k-proj: k(0,2..7) after kstg blocks land (needed slot 4g);
            # k(1,0..1) late hp0, k(1,2..7) early hp1; k(2,0) late hp1,
            # k(2,1..7) early hp2; k(3,0) late hp2, k(3,1..7) early hp3.
            karr = {2: 3, 3: 5, 4: 8, 5: 10, 6: 12, 7: 14}
            for g in range(2, 8):
                a, b = k_group_parts(0, g)
                s = max(karr[g], 4 * g - 9)
                at(0, 0, s, a)
                at(0, 0, s + 1, b)
            # rotated hp1 consumes k(1,*) groups in order 2,3,4..7,0,1:
            # the first two must finish in hp0, the rest ride hp1 odd slots
            for g in range(2, 4):
                a, b = k_group_parts(1, g)
                at(0, 0, 12 + 2 * g, a)         # g=2 -> 16,17; g=3 -> 18,19
                at(0, 0, 13 + 2 * g, b)
            for i, g in enumerate([4, 5, 6, 7]):
                a, b = k_group_parts(1, g)
                at(0, 1, 4 * i + 1, a)
                at(0, 1, 4 * i + 3, b)
            for i, g in enumerate([0, 1]):      # deadlines: slots 20 / 24
                a, b = k_group_parts(1, g)
                at(0, 1, 4 * i + 2, a)
                at(0, 1, 4 * i + 4, b)
            a, b = k_group_parts(2, 0)
            at(0, 1, 24, a)
            at(0, 1, 26, b)
            for g in range(1, 8):
                a, b = k_group_parts(2, g)
                at(0, 2, 2 * g - 2, a)
                at(0, 2, 2 * g - 1, b)
            a, b = k_group_parts(3, 0)
            at(0, 2, 26, a)
            at(0, 2, 27, b)
            for g in range(1, 8):
                a, b = k_group_parts(3, g)
                at(0, 3, 2 * g - 2, a)
                at(0, 3, 2 * g - 1, b)

            # q-proj: q(0,1) late hp0, q(0,2) mid hp1, q(0,3) mid hp2;
            # q(1,*) during hp3 (interleaved with k(3,*))
            a, b = q_group_parts(0, 1)
            at(0, 0, 29, a)
            at(0, 0, 30, b)
            a, b = q_group_parts(0, 2)
            at(0, 1, 25, a)
            at(0, 1, 27, b)
            a, b = q_group_parts(0, 3)
            at(0, 2, 20, [a, b])
            # q(1,0) must land in mg0-hp3 (mg1-hp0 scores emitted at its
            # last slot); q(1,1..3) ride in mg1-hp0 (needed at hp1/2/3)
            a, b = q_group_parts(1, 0)
            at(0, 3, 15, a)
            at(0, 3, 16, b)
            for co in range(1, 4):
                a, b = q_group_parts(1, co)
                at(1, 0, 8 * co - 6, [a, b])

            # mg1 fillers: mg0 out-proj during hp0; partial out-proj placed
            # past the ~5us normalization latency of the previous head-pair
            # so its matmuls never head-block the PE queue
            for co in range(4):
                for i, fn in enumerate(outproj0_parts(co)):
                    at(1, 0, 8 * co + 2 * i + 1, fn)
            for hp in range(3):
                for co in range(4):
                    at(1, hp + 1, 8 + 4 * co, outproj1_partial(hp, co))

            # ---------------- PE warm-up ----------------
            # ~5us of dummy matmuls while the input DMA streams: drives the
            # HAM clock-gate to 8/8 so the first projections run at 2.4GHz.
            # Alternate psum pools for 4-deep rotation (back-to-back MMs).
            for i in range(12):
                if i % 2 == 0:
                    wup = ps_proj.tile([128, 512], F32, tag="pp", name="wup")
                    dst = wup[0:64, :]
                else:
                    wup = ps_sab.tile([128, 2, 512], F32, tag="sab", name="wup")
                    dst = wup[0:64, 0, :]
                nc.tensor.matmul(dst, vha[0:64, 0, 0:64],
                                 vha[0:64, 0, 0:512], start=True, stop=True)

            # ---------------- upfront projections ----------------
            for fn in q_group_parts(0, 0):
                fn()
            # k(0,0) in 256-key halves, tracking the split block-0 DMA
            for half in range(2):
                pp = ps_proj.tile([128, 512], F32, tag="pp", name="pp")
                for ci in range(4):
                    nc.tensor.matmul(pp[:, ts(half, 256)],
                                     w0_sb[:, 1, ci, :],
                                     kstg[:, 0, ci, ts(half, 256)],
                                     start=(ci == 0), stop=(ci == 3))
                nc.vector.tensor_scalar(
                    out=kht[:, 0, ts(half, 256)], in0=pp[:, ts(half, 256)],
                    scalar1=bqk[:, 4:5], scalar2=None, op0=ADD)
            for fn in k_group_parts(0, 1):
                fn()
            for sc in range(3):
                v_hp0(sc)()

            # ---------------- attention (flat, scores one chunk ahead) ----
            NT = NQG * 4 * NSC
            ROT = {(0, 1): 10}   # mg0-hp1: slot s handles chunk 10+s mod 32

            def decode(t):
                return t // (4 * NSC), (t // NSC) % 4, t % NSC

            def chunk_of(mg, hp, s):
                return (s + ROT.get((mg, hp), 0)) % NSC

            def emit_scores(t):
                mg, hp, s = decode(t)
                sc = chunk_of(mg, hp, s)
                sab = ps_sab.tile([128, 2, 512], F32, tag="sab")
                nc.tensor.matmul(sab[:, 0, :],
                                 kht[0:64, hp, ts(sc, 128)],
                                 qht[0:64, hp, ts(mg, 512)],
                                 start=True, stop=True, tile_position=(0, 0))
                nc.tensor.matmul(sab[:, 1, :],
                                 kht[64:128, hp, ts(sc, 128)],
                                 qht[64:128, hp, ts(mg, 512)],
                                 start=True, stop=True, tile_position=(64, 0))
                return sab

            pab_hist = {}
            pv_of = {}

            def emit_pv(tt):
                mg_, hp_, s_ = decode(tt)
                ch_ = chunk_of(mg_, hp_, s_)
                hA_, hB_ = 2 * hp_, 2 * hp_ + 1
                pvt = pv_of.pop(tt)
                pab_ = pab_hist.pop(tt)
                nc.tensor.matmul(pvt[:, 0, :],
                                 vha[:, ch_, hA_ * 65: hA_ * 65 + 65],
                                 pab_[:, 0, :],
                                 start=(s_ == 0), stop=(s_ == NSC - 1))
                nc.tensor.matmul(pvt[:, 1, :],
                                 vha[:, ch_, hB_ * 65: hB_ * 65 + 65],
                                 pab_[:, 1, :],
                                 start=(s_ == 0), stop=(s_ == NSC - 1))
                return pvt

            def emit_evac(pv_):
                # only the PSUM evacuation must precede the next
                # head-pair's first PV (WAR on the pv banks)
                pvs = pvs_p.tile([65, 2, 512], F32, tag="pvs")
                nc.vector.tensor_copy(pvs[:], pv_[:])
                return pvs

            def emit_norm_rest(mg_, hp_, pvs):
                # SBUF-side normalization: lagged 2 slots so the boundary
                # fillers' DVE evacuations are not queued behind it
                hA_, hB_ = 2 * hp_, 2 * hp_ + 1
                rs = rv_p.tile([1, 2, 512], F32, tag="rs")
                nc.vector.tensor_copy(rs[:], pvs[64:65, :, :])
                rv = rv_p.tile([1, 2, 512], F32, tag="rv")
                nc.vector.reciprocal_approx_fast(out=rv[:], in_=rs[:])
                rrep = pvs_p.tile([64, 2, 512], F32, tag="pvs", name="rrep")
                nc.gpsimd.partition_broadcast(rrep[:], rv[:])
                nc.vector.tensor_tensor(
                    out=att[:, hA_:hA_ + 2, ts(mg_, 512)], in0=pvs[0:64, :, :],
                    in1=rrep[:], op=MULT)

            def emit_norm(mg_, hp_, pv_, is_last):
                hA_, hB_ = 2 * hp_, 2 * hp_ + 1
                if not is_last:
                    emit_norm_rest(mg_, hp_, emit_evac(pv_))
                else:
                    # tail fast path: rowsum straight from PSUM,
                    # reciprocal, PE ones-broadcast into PSUM, multiply.
                    # pv evacuation runs on the (now idle) ACT engine in
                    # parallel with the DVE rowsum/reciprocal chain.
                    rs = rv_p.tile([1, 2, 512], F32, tag="rs")
                    nc.vector.tensor_copy(rs[:], pv_[64:65, :, :])
                    pvs = pvs_p.tile([65, 2, 512], F32, tag="pvs")
                    nc.scalar.copy(pvs[:], pv_[:])
                    rv = rv_p.tile([1, 2, 512], F32, tag="rv")
                    nc.vector.reciprocal_approx_fast(out=rv[:], in_=rs[:])
                    rvb = pvs_p.tile([1, 2, 512], BF16, tag="pvs", name="rvb")
                    nc.vector.tensor_copy(rvb[:], rv[:])
                    bc = ps_sab.tile([64, 2, 512], F32, tag="sab", name="bc")
                    for i in range(2):
                        nc.tensor.matmul(bc[:, i, :], ones1[:, :],
                                         rvb[:, i, :], start=True, stop=True)
                    nc.vector.tensor_tensor(
                        out=att[:, hA_:hA_ + 2, ts(mg_, 512)],
                        in0=pvs[0:64, :, :], in1=bc[:], op=MULT)

            # PV emission lags one slot: slot t emits [exp(t), scores(t+1),
            # pv(t-1), fillers].  At head-pair boundaries the new
            # accumulator's first PV (which WAR-waits the previous pv's
            # DVE evacuation) then sits BEHIND the next chunks' scores in
            # the in-order PE queue instead of stalling them.
            sab_cur = emit_scores(0)
            pv_cur = ps_pv.tile([65, 2, 512], F32, tag="pv")
            norm_pending = None
            for t in range(NT):
                mg, hp, sc = decode(t)
                last = (t == NT - 1)
                pab = pab_p.tile([128, 2, 512], BF16, tag="pab")
                pab_hist[t] = pab
                nc.scalar.activation(pab[:], sab_cur[:], EXP, scale=SCALE)
                if not last:
                    sab_cur = emit_scores(t + 1)
                if t >= 1:
                    pv_done = emit_pv(t - 1)
                    if sc == 0:
                        # t-1 closed the previous head-pair: evacuate its
                        # pv now, allocate this head-pair's accumulator;
                        # the rest of the normalization lags 2 slots
                        pmg, php, _ = decode(t - 1)
                        norm_pending = (pmg, php, emit_evac(pv_done))
                        pv_cur = ps_pv.tile([65, 2, 512], F32, tag="pv")
                if sc == 2 and norm_pending is not None:
                    emit_norm_rest(*norm_pending)
                    norm_pending = None
                pv_of[t] = pv_cur
                for fn in pending.pop((mg, hp, sc), ()):
                    fn()
            pv_done = emit_pv(NT - 1)
            emit_norm(NQG - 1, 3, pv_done, is_last=True)

            # ---------------- tail: last head-pair partials ----------------
            for co in range(4):
                outproj1_partial(3, co)()

            assert not pending, f"unconsumed fillers: {list(pending)}"

    nc.compile()
    return nc


def _get_nc():
    if "nc" not in _STATE:
        _STATE["nc"] = _build()
    return _STATE["nc"]


def _bf16(x):
    return np.ascontiguousarray(x.astype(ml_dtypes.bfloat16))


def _pmajor(xT):
    # xT: [E, Lc] (feature-major) -> [128, Lc//512, 4, 512]
    # out[p, blk, ci, l] = xT[ci*128 + p, blk*512 + l]
    E_, Lc = xT.shape
    return np.ascontiguousarray(
        xT.reshape(4, 128, Lc // 512, 512).transpose(1, 2, 0, 3))


def _wmajor(WT):
    # WT: [E, E] -> [128, 4, 512]; out[p, ci, o] = WT[ci*128 + p, o]
    return np.ascontiguousarray(WT.reshape(4, 128, 512).transpose(1, 0, 2))


def _shard(inputs):
    q = np.asarray(inputs["q"], dtype=np.float32)
    k = np.asarray(inputs["k"], dtype=np.float32)
    v = np.asarray(inputs["v"], dtype=np.float32)
    WqT = np.asarray(inputs["Wq"], np.float32).T
    WkT = np.asarray(inputs["Wk"], np.float32).T
    WvT = np.asarray(inputs["Wv"], np.float32).T
    WoT = np.asarray(inputs["Wo"], np.float32).T
    bq = np.asarray(inputs["bq"], np.float32)
    bk = np.asarray(inputs["bk"], np.float32)

    wqf = _bf16(_wmajor(WqT))
    wkf = _bf16(_wmajor(WkT))
    wvf = _bf16(_wmajor(WvT))
    # w0: first-128-output-cols slices of wq/wk/wv: [128, 3, 4, 128]
    w0 = _bf16(np.stack([wqf[:, :, 0:128], wkf[:, :, 0:128],
                         wvf[:, :, 0:128]], axis=1))
    # wo: [64, 8, 512]; [d, h, o] = WoT[h*64+d, o]
    wof = _bf16(WoT.reshape(8, 64, 512).transpose(1, 0, 2))
    bqk = np.ascontiguousarray(
        np.concatenate([bq.reshape(4, 128).T, bk.reshape(4, 128).T], axis=1))

    kt = [_bf16(_pmajor(k[b].T)) for b in range(B)]
    vt = [_bf16(_pmajor(v[b].T)) for b in range(B)]

    in_maps = []
    for c in range(N_CORES):
        b, j = divmod(c, N_CORES // B)
        in_maps.append({
            "qt": _bf16(_pmajor(q[b, j * LLOC:(j + 1) * LLOC].T)),
            "kt": kt[b],
            "vt": vt[b],
            "w0": w0, "wqf": wqf, "wkf": wkf, "wvf": wvf, "wof": wof,
            "bqk": bqk,
        })
    return in_maps


def _run(inputs, trace=False):
    nc = _get_nc()
    in_maps = _shard(inputs)
    res = run_bass_kernel_spmd(nc, in_maps, core_ids=list(range(N_CORES)),
                               trace=trace)
    # v-bias commutes through attention (rows of P sum to 1 after
    # normalization): fold Wo @ bv into the output bias, added on host.
    Wo = np.asarray(inputs["Wo"], np.float32)
    bo_eff = (np.asarray(inputs["bo"], np.float32)
              + Wo @ np.asarray(inputs["bv"], np.float32))
    out = np.empty((B, L, E), np.float32)
    for c in range(N_CORES):
        b, j = divmod(c, N_CORES // B)
        out[b, j * LLOC:(j + 1) * LLOC] = \
            res.results[c]["out"].astype(np.float32).T + bo_eff
    return out, res


def kernel(**inputs) -> np.ndarray:
    return _run(inputs)[0]
